# revision 15
# baseline (speedup 1.0000x reference)
"""Trainium2 Bass kernel: multi-head attention (B=4, N=1024, D=1024, H=16)
distributed over 8 NeuronCores.

kernel(**inputs) takes the FULL inputs (x, w_qkv, w_out, b_out) as numpy
arrays, shards them (batch, query-half) -> one core each, runs an SPMD Bass
kernel on cores 0-7 and reassembles the full [4, 1024, 1024] fp32 output.

Per-core layout (all bf16 compute on the PE, fp32 PSUM):
  - x^T for the core's batch arrives token-ROTATED so that the core's own
    512 queries are always tokens 0:512 (one SPMD graph for all cores;
    attention over keys is permutation invariant).
  - scores PSUM tiles are [128, 2, 512] (two banks) so the ACT engine exps
    1024 columns per instruction instead of 512.
  - each PV stationary tile is [ones(64 cols) | v(64 dims)], so the PV
    matmul replicates the softmax denominator across PSUM partitions
    0:64 for free (the custom-DVE reciprocal ignores a nonzero input
    partition offset on HW, so the denominator must sit at base 0);
    normalization is then one reciprocal_approx_fast [64,512] + one
    tensor_mul per head on the DVE (no broadcast matmul, no slow
    InstReciprocal).
  - the output projection starts its PSUM accumulation directly and the
    bias lands during the DVE eviction, from a bias row broadcast across
    partitions once at startup.
  - K-proj / out-proj matmuls are emitted in same-stationary pairs so the
    weight loads amortize across two moving streams.
"""

import numpy as np
import concourse.bacc as bacc
import concourse.mybir as mybir
import concourse.tile as tile

dt = mybir.dt
F32, BF16 = dt.float32, dt.bfloat16

B, N, D = 4, 1024, 1024
H, DH = 16, 64
NQ = 512            # queries per core
P = 128
DC = D // P         # 8 contraction chunks
NT = N // P         # 8 key-token tiles
ET = 8              # feature tiles per q/k section
SCALE = DH ** -0.5
AF = mybir.ActivationFunctionType


def _build_nc():
    nc = bacc.Bacc("TRN2", target_bir_lowering=False, debug=False)
    xkT = nc.dram_tensor("xkT", [D, N], BF16, kind="ExternalInput")
    wqkvT = nc.dram_tensor("wqkvT", [D, 3 * D], BF16, kind="ExternalInput")
    woutT = nc.dram_tensor("woutT", [D, D], BF16, kind="ExternalInput")
    bout = nc.dram_tensor("bout", [1, D], BF16, kind="ExternalInput")
    y = nc.dram_tensor("y", [NQ, D], F32, kind="ExternalOutput")

    with tile.TileContext(nc) as tc:
        with (
            tc.tile_pool(name="const", bufs=1) as cp,
            tc.tile_pool(name="work", bufs=2) as wp,
            tc.tile_pool(name="ps", bufs=1, space="PSUM") as pp,
        ):
            xk_sb = cp.tile([P, DC, N], BF16)
            wqkv_sb = cp.tile([P, DC, 3 * D], BF16)
            wout_sb = cp.tile([P, DC, D], BF16)
            bout_sb = cp.tile([1, D], BF16)
            q_sb = cp.tile([P, ET, NQ], BF16)     # [dh, et, q]
            k_sb = cp.tile([P, ET, N], BF16)      # [dh, et, tok]
            v_sb = cp.tile([P, NT, H, P], BF16)   # [tok, c, h, 64v+64ones]
            aT_sb = cp.tile([P, DC, NQ], BF16)    # [feat, c, q]
            ones128 = cp.tile([1, P], BF16)

            # ones block of every PV stationary tile + bias-broadcast row.
            # First DVE work; finishes during the DMA prologue.
            nc.vector.memset(v_sb[:, :, :, 0:DH], 1.0)
            nc.gpsimd.memset(ones128, 1.0)

            # ---- input DMA, chunked in consumption order ----
            # x chunks feed everything; w chunks ordered by first use.
            # x + (q-et0|k-et0) weight columns interleaved per chunk so the
            # first projection chains can start as soon as chunk 0 lands
            for c in range(DC):
                nc.sync.dma_start(xk_sb[:, c, :], xkT.ap()[c * P:(c + 1) * P, :])
                nc.sync.dma_start(
                    wqkv_sb[:, c, 0:2 * D].rearrange("p (g e) -> p g e", g=2)[:, :, 0:P],
                    wqkvT.ap()[c * P:(c + 1) * P, 0:2 * D]
                    .rearrange("p (g e) -> p g e", g=2)[:, :, 0:P],
                )
            # v columns, first half (heads 0-7)
            for c in range(DC):
                nc.sync.dma_start(wqkv_sb[:, c, 2 * D:2 * D + NQ],
                                  wqkvT.ap()[c * P:(c + 1) * P, 2 * D:2 * D + NQ])
            # q-et1..7 + k-et1..7 columns
            for c in range(DC):
                nc.sync.dma_start(
                    wqkv_sb[:, c, 0:2 * D].rearrange("p (g e) -> p g e", g=2)[:, :, P:D],
                    wqkvT.ap()[c * P:(c + 1) * P, 0:2 * D]
                    .rearrange("p (g e) -> p g e", g=2)[:, :, P:D],
                )
            # v columns, second half (heads 8-15)
            for c in range(DC):
                nc.sync.dma_start(wqkv_sb[:, c, 2 * D + NQ:3 * D],
                                  wqkvT.ap()[c * P:(c + 1) * P, 2 * D + NQ:3 * D])
            nc.sync.dma_start(bout_sb[:, :], bout.ap()[:, :])
            for c in range(DC):
                nc.sync.dma_start(wout_sb[:, c, :], woutT.ap()[c * P:(c + 1) * P, :])

            # ---- projection emitters ----
            def q_proj(et):
                q_ps = pp.tile([P, NQ], F32, tag="proj", bufs=3, name=f"qps{et}")
                for c in range(DC):
                    nc.tensor.matmul(
                        q_ps[:, :],
                        lhsT=wqkv_sb[:, c, et * P:(et + 1) * P],
                        rhs=xk_sb[:, c, 0:NQ],
                        start=(c == 0), stop=(c == DC - 1),
                    )
                nc.vector.tensor_copy(q_sb[:, et, :], q_ps[:, :])

            def q_steps(et):
                state = {}
                def step(c):
                    if c == 0:
                        state["ps"] = pp.tile([P, NQ], F32, tag="proj", bufs=3,
                                              name=f"qps{et}")
                    nc.tensor.matmul(
                        state["ps"][:, :],
                        lhsT=wqkv_sb[:, c, et * P:(et + 1) * P],
                        rhs=xk_sb[:, c, 0:NQ],
                        start=(c == 0), stop=(c == DC - 1),
                    )
                    if c == DC - 1:
                        nc.vector.tensor_copy(q_sb[:, et, :], state["ps"][:, :])
                return [lambda c=c: step(c) for c in range(DC)]

            # K-proj: both query-halves j=0,1 share the stationary w tile per
            # chunk, emitted back-to-back so the weight load amortizes.
            def k_steps(et):
                state = {}
                def step(c, j):
                    if c == 0 and j == 0:
                        state[0] = pp.tile([P, NQ], F32, tag="proj", bufs=3,
                                           name=f"kps{et}_0")
                        state[1] = pp.tile([P, NQ], F32, tag="proj", bufs=3,
                                           name=f"kps{et}_1")
                    nc.tensor.matmul(
                        state[j][:, :],
                        lhsT=wqkv_sb[:, c, D + et * P:D + (et + 1) * P],
                        rhs=xk_sb[:, c, j * NQ:(j + 1) * NQ],
                        start=(c == 0), stop=(c == DC - 1),
                    )
                    if c == DC - 1:
                        nc.vector.tensor_copy(k_sb[:, et, j * NQ:(j + 1) * NQ],
                                              state[j][:, :])
                return [lambda c=c, j=j: step(c, j) for c in range(DC) for j in (0, 1)]

            def v_steps(nt, j):
                state = {}
                def step(c):
                    if c == 0:
                        state["ps"] = pp.tile([P, NQ], F32, tag="proj", bufs=3,
                                              name=f"vps{nt}_{j}")
                    nc.tensor.matmul(
                        state["ps"][:, :],
                        lhsT=xk_sb[:, c, nt * P:(nt + 1) * P],
                        rhs=wqkv_sb[:, c, 2 * D + j * NQ:2 * D + (j + 1) * NQ],
                        start=(c == 0), stop=(c == DC - 1),
                    )
                    if c == DC - 1:
                        nc.vector.tensor_copy(
                            v_sb[:, nt, j * 8:(j + 1) * 8, DH:P],
                            state["ps"][:, :].rearrange("p (h d) -> p h d", h=8),
                        )
                return [lambda c=c: step(c) for c in range(DC)]

            # ---- filler queue: projection matmuls threaded between the
            # attention matmuls so the PE never idles while ACT exps. ----
            filler_units = []
            for et in range(1, 5):
                filler_units.append((2 * et, q_steps(et)))
                filler_units.append((2 * et, k_steps(et)))
            for nt in range(NT):
                filler_units.append((8, v_steps(nt, 1)))
            for et in range(5, ET):
                filler_units.append((2 * et, q_steps(et)))
                filler_units.append((2 * et, k_steps(et)))
            filler_steps = [(dl, s) for dl, steps in filler_units for s in steps]
            fill_pos = 0

            def flush_fillers(h):
                nonlocal fill_pos
                while fill_pos < len(filler_steps) and filler_steps[fill_pos][0] <= h:
                    filler_steps[fill_pos][1]()
                    fill_pos += 1

            def pop_filler(n):
                nonlocal fill_pos
                k = 0
                while k < n and fill_pos < len(filler_steps):
                    filler_steps[fill_pos][1]()
                    fill_pos += 1
                    k += 1

            # ---- one attention head ----
            def head(h, fill_per_batch):
                t, r = h // 2, (h % 2) * DH
                flush_fillers(h)
                pT = wp.tile([P, NT, NQ], BF16, tag="pT", bufs=2, name=f"pT{h}")
                for cp2 in range(NT // 2):
                    s_ps = pp.tile([P, 2, NQ], F32, tag="s", bufs=2,
                                   name=f"s{h}_{cp2}")
                    for j in range(2):
                        ct = 2 * cp2 + j
                        nc.tensor.matmul(
                            s_ps[:, j, :],
                            lhsT=k_sb[r:r + DH, t, ct * P:(ct + 1) * P],
                            rhs=q_sb[r:r + DH, t, :],
                            start=True, stop=True,
                        )
                    nc.scalar.activation(pT[:, 2 * cp2:2 * cp2 + 2, :],
                                         s_ps[:, :, :], AF.Exp, scale=SCALE)
                    pop_filler(fill_per_batch[cp2])
                flush_fillers(h + 0.5)
                pv_ps = pp.tile([P, NQ], F32, tag="pv", bufs=1, name=f"pv{h}")
                for c in range(NT):
                    nc.tensor.matmul(
                        pv_ps[:, :],
                        lhsT=v_sb[:, c, h, :],
                        rhs=pT[:, c, :],
                        start=(c == 0), stop=(c == NT - 1),
                    )
                # rows 0:64 of pv_ps hold the softmax denominator replicated
                # 64x (ones block leads the stationary tile -- the custom DVE
                # reciprocal requires its input at partition base 0 on HW);
                # rows 64:128 are the PV values. Normalize with two DVE ops.
                srec = wp.tile([DH, NQ], F32, tag="srec", bufs=2, name=f"sr{h}")
                nc.vector.reciprocal_approx_fast(srec[:, :], pv_ps[0:DH, :])
                nc.vector.tensor_mul(aT_sb[r:r + DH, t, :], pv_ps[DH:P, :],
                                     srec[:, :])

            # ---- schedule ----
            bias_sb = cp.tile([P, 2, NQ], F32)
            b_ps = pp.tile([P, 2, NQ], F32, tag="s", bufs=2, name="biasps")
            for j in range(2):
                nc.tensor.matmul(b_ps[:, j, :], lhsT=ones128[:, :],
                                 rhs=bout_sb[:, j * NQ:(j + 1) * NQ],
                                 start=True, stop=True)
            nc.vector.tensor_copy(bias_sb[:, :, :], b_ps[:, :, :])
            q_proj(0)
            for s in k_steps(0):
                s()
            for nt in range(NT):
                for s in v_steps(nt, 0):
                    s()
            pacing = {c: 4 for c in range(NT // 2)}
            pacing_late = {c: 3 for c in range(NT // 2)}
            for h in range(H):
                head(h, pacing if h < 8 else pacing_late)

            # ---- output projection: 4 token-tiles of [128, 1024] ----
            for tt in range(NQ // P):
                y_ps = pp.tile([P, 2, NQ], F32, tag="s", bufs=2, name=f"yps{tt}")
                for c in range(DC):
                    for j in range(2):
                        nc.tensor.matmul(
                            y_ps[:, j, :],
                            lhsT=aT_sb[:, c, tt * P:(tt + 1) * P],
                            rhs=wout_sb[:, c, j * NQ:(j + 1) * NQ],
                            start=(c == 0), stop=(c == DC - 1),
                            skip_group_check=True,
                        )
                y_sb = wp.tile([P, D], F32, tag="y_sb", bufs=2, name=f"ysb{tt}")
                if tt < NQ // P - 1:
                    nc.vector.tensor_add(
                        y_sb[:, :].rearrange("p (g e) -> p g e", g=2),
                        y_ps[:, :, :], bias_sb[:, :, :])
                    nc.sync.dma_start(y.ap()[tt * P:(tt + 1) * P, :], y_sb[:, :])
                else:
                    for j in range(2):
                        nc.vector.tensor_add(y_sb[:, j * NQ:(j + 1) * NQ],
                                             y_ps[:, j, :], bias_sb[:, j, :])
                        nc.sync.dma_start(
                            y.ap()[tt * P:(tt + 1) * P, j * NQ:(j + 1) * NQ],
                            y_sb[:, j * NQ:(j + 1) * NQ])
    nc.compile()
    return nc


def _make_in_maps(x, w_qkv, w_out, b_out):
    import ml_dtypes
    bf = ml_dtypes.bfloat16
    wqkvT = np.ascontiguousarray(w_qkv.astype(bf).T)
    woutT = np.ascontiguousarray(w_out.astype(bf).T)
    boutr = b_out.astype(bf).reshape(1, D)
    in_maps = []
    for core in range(8):
        b, half = core // 2, core % 2
        xT = x[b].astype(bf).T
        if half:
            xT = np.concatenate([xT[:, NQ:], xT[:, :NQ]], axis=1)
        in_maps.append({
            "xkT": np.ascontiguousarray(xT),
            "wqkvT": wqkvT,
            "woutT": woutT,
            "bout": boutr,
        })
    return in_maps


def _assemble(results):
    y = np.empty((B, N, D), dtype=np.float32)
    for core in range(8):
        b, half = core // 2, core % 2
        y[b, half * NQ:(half + 1) * NQ, :] = results[core]["y"]
    return y


_NC_CACHE = {}


def kernel(x, w_qkv, w_out, b_out):
    import numpy as _np
    from concourse.bass_utils import run_bass_kernel_spmd
    if "nc" not in _NC_CACHE:
        _NC_CACHE["nc"] = _build_nc()
    nc = _NC_CACHE["nc"]
    in_maps = _make_in_maps(_np.asarray(x), _np.asarray(w_qkv),
                            _np.asarray(w_out), _np.asarray(b_out))
    res = run_bass_kernel_spmd(nc, in_maps, list(range(8)))
    return _assemble(res.results)


# revision 16
# speedup vs baseline: 1.0022x; 1.0022x over previous
"""Trainium2 Bass kernel: multi-head attention (B=4, N=1024, D=1024, H=16)
distributed over 8 NeuronCores.

kernel(**inputs) takes the FULL inputs (x, w_qkv, w_out, b_out) as numpy
arrays, shards them (batch, query-half) -> one core each, runs an SPMD Bass
kernel on cores 0-7 and reassembles the full [4, 1024, 1024] fp32 output.

Per-core layout (all bf16 compute on the PE, fp32 PSUM):
  - x^T for the core's batch arrives token-ROTATED so that the core's own
    512 queries are always tokens 0:512 (one SPMD graph for all cores;
    attention over keys is permutation invariant).
  - scores PSUM tiles are [128, 2, 512] (two banks) so the ACT engine exps
    1024 columns per instruction instead of 512.
  - each PV stationary tile is [ones(64 cols) | v(64 dims)], so the PV
    matmul replicates the softmax denominator across PSUM partitions
    0:64 for free (the custom-DVE reciprocal ignores a nonzero input
    partition offset on HW, so the denominator must sit at base 0);
    normalization is then one reciprocal_approx_fast [64,512] + one
    tensor_mul per head on the DVE (no broadcast matmul, no slow
    InstReciprocal).
  - the output projection starts its PSUM accumulation directly and the
    bias lands during the DVE eviction, from a bias row broadcast across
    partitions once at startup.
  - K-proj / out-proj matmuls are emitted in same-stationary pairs so the
    weight loads amortize across two moving streams.
"""

import numpy as np
import concourse.bacc as bacc
import concourse.mybir as mybir
import concourse.tile as tile

dt = mybir.dt
F32, BF16 = dt.float32, dt.bfloat16

B, N, D = 4, 1024, 1024
H, DH = 16, 64
NQ = 512            # queries per core
P = 128
DC = D // P         # 8 contraction chunks
NT = N // P         # 8 key-token tiles
ET = 8              # feature tiles per q/k section
SCALE = DH ** -0.5
AF = mybir.ActivationFunctionType


def _build_nc():
    nc = bacc.Bacc("TRN2", target_bir_lowering=False, debug=False)
    xkT = nc.dram_tensor("xkT", [D, N], BF16, kind="ExternalInput")
    wqkvT = nc.dram_tensor("wqkvT", [D, 3 * D], BF16, kind="ExternalInput")
    woutT = nc.dram_tensor("woutT", [D, D], BF16, kind="ExternalInput")
    bout = nc.dram_tensor("bout", [1, D], BF16, kind="ExternalInput")
    y = nc.dram_tensor("y", [NQ, D], F32, kind="ExternalOutput")

    with tile.TileContext(nc) as tc:
        with (
            tc.tile_pool(name="const", bufs=1) as cp,
            tc.tile_pool(name="work", bufs=2) as wp,
            tc.tile_pool(name="ps", bufs=1, space="PSUM") as pp,
        ):
            xk_sb = cp.tile([P, DC, N], BF16)
            wqkv_sb = cp.tile([P, DC, 3 * D], BF16)
            wout_sb = cp.tile([P, DC, D], BF16)
            bout_sb = cp.tile([1, D], BF16)
            q_sb = cp.tile([P, ET, NQ], BF16)     # [dh, et, q]
            k_sb = cp.tile([P, ET, N], BF16)      # [dh, et, tok]
            v_sb = cp.tile([P, NT, H, P], BF16)   # [tok, c, h, 64v+64ones]
            aT_sb = cp.tile([P, DC, NQ], BF16)    # [feat, c, q]
            ones128 = cp.tile([1, P], BF16)

            # ones block of every PV stationary tile + bias-broadcast row.
            # First DVE work; finishes during the DMA prologue.
            nc.vector.memset(v_sb[:, :, :, 0:DH], 1.0)
            nc.vector.memset(ones128, 1.0)

            # ---- input DMA, chunked in consumption order ----
            # x chunks feed everything; w chunks ordered by first use.
            # x + (q-et0|k-et0) weight columns interleaved per chunk so the
            # first projection chains can start as soon as chunk 0 lands
            for c in range(DC):
                nc.sync.dma_start(xk_sb[:, c, :], xkT.ap()[c * P:(c + 1) * P, :])
                nc.sync.dma_start(
                    wqkv_sb[:, c, 0:2 * D].rearrange("p (g e) -> p g e", g=2)[:, :, 0:P],
                    wqkvT.ap()[c * P:(c + 1) * P, 0:2 * D]
                    .rearrange("p (g e) -> p g e", g=2)[:, :, 0:P],
                )
            # v columns, first half (heads 0-7)
            for c in range(DC):
                nc.sync.dma_start(wqkv_sb[:, c, 2 * D:2 * D + NQ],
                                  wqkvT.ap()[c * P:(c + 1) * P, 2 * D:2 * D + NQ])
            # q-et1..7 + k-et1..7 columns
            for c in range(DC):
                nc.sync.dma_start(
                    wqkv_sb[:, c, 0:2 * D].rearrange("p (g e) -> p g e", g=2)[:, :, P:D],
                    wqkvT.ap()[c * P:(c + 1) * P, 0:2 * D]
                    .rearrange("p (g e) -> p g e", g=2)[:, :, P:D],
                )
            # v columns, second half (heads 8-15)
            for c in range(DC):
                nc.sync.dma_start(wqkv_sb[:, c, 2 * D + NQ:3 * D],
                                  wqkvT.ap()[c * P:(c + 1) * P, 2 * D + NQ:3 * D])
            nc.sync.dma_start(bout_sb[:, :], bout.ap()[:, :])
            for c in range(DC):
                nc.sync.dma_start(wout_sb[:, c, :], woutT.ap()[c * P:(c + 1) * P, :])

            # ---- projection emitters ----
            def q_proj(et):
                q_ps = pp.tile([P, NQ], F32, tag="proj", bufs=3, name=f"qps{et}")
                for c in range(DC):
                    nc.tensor.matmul(
                        q_ps[:, :],
                        lhsT=wqkv_sb[:, c, et * P:(et + 1) * P],
                        rhs=xk_sb[:, c, 0:NQ],
                        start=(c == 0), stop=(c == DC - 1),
                    )
                nc.vector.tensor_copy(q_sb[:, et, :], q_ps[:, :])

            def q_steps(et):
                state = {}
                def step(c):
                    if c == 0:
                        state["ps"] = pp.tile([P, NQ], F32, tag="proj", bufs=3,
                                              name=f"qps{et}")
                    nc.tensor.matmul(
                        state["ps"][:, :],
                        lhsT=wqkv_sb[:, c, et * P:(et + 1) * P],
                        rhs=xk_sb[:, c, 0:NQ],
                        start=(c == 0), stop=(c == DC - 1),
                    )
                    if c == DC - 1:
                        nc.vector.tensor_copy(q_sb[:, et, :], state["ps"][:, :])
                return [lambda c=c: step(c) for c in range(DC)]

            # K-proj: both query-halves j=0,1 share the stationary w tile per
            # chunk, emitted back-to-back so the weight load amortizes.
            def k_steps(et):
                state = {}
                def step(c, j):
                    if c == 0 and j == 0:
                        state[0] = pp.tile([P, NQ], F32, tag="proj", bufs=3,
                                           name=f"kps{et}_0")
                        state[1] = pp.tile([P, NQ], F32, tag="proj", bufs=3,
                                           name=f"kps{et}_1")
                    nc.tensor.matmul(
                        state[j][:, :],
                        lhsT=wqkv_sb[:, c, D + et * P:D + (et + 1) * P],
                        rhs=xk_sb[:, c, j * NQ:(j + 1) * NQ],
                        start=(c == 0), stop=(c == DC - 1),
                    )
                    if c == DC - 1:
                        nc.vector.tensor_copy(k_sb[:, et, j * NQ:(j + 1) * NQ],
                                              state[j][:, :])
                return [lambda c=c, j=j: step(c, j) for c in range(DC) for j in (0, 1)]

            def v_steps(nt, j):
                state = {}
                def step(c):
                    if c == 0:
                        state["ps"] = pp.tile([P, NQ], F32, tag="proj", bufs=3,
                                              name=f"vps{nt}_{j}")
                    nc.tensor.matmul(
                        state["ps"][:, :],
                        lhsT=xk_sb[:, c, nt * P:(nt + 1) * P],
                        rhs=wqkv_sb[:, c, 2 * D + j * NQ:2 * D + (j + 1) * NQ],
                        start=(c == 0), stop=(c == DC - 1),
                    )
                    if c == DC - 1:
                        nc.vector.tensor_copy(
                            v_sb[:, nt, j * 8:(j + 1) * 8, DH:P],
                            state["ps"][:, :].rearrange("p (h d) -> p h d", h=8),
                        )
                return [lambda c=c: step(c) for c in range(DC)]

            # ---- filler queue: projection matmuls threaded between the
            # attention matmuls so the PE never idles while ACT exps. ----
            filler_units = []
            for et in range(1, 5):
                filler_units.append((2 * et, q_steps(et)))
                filler_units.append((2 * et, k_steps(et)))
            for nt in range(NT):
                filler_units.append((8, v_steps(nt, 1)))
            for et in range(5, ET):
                filler_units.append((2 * et, q_steps(et)))
                filler_units.append((2 * et, k_steps(et)))
            filler_steps = [(dl, s) for dl, steps in filler_units for s in steps]
            fill_pos = 0

            def flush_fillers(h):
                nonlocal fill_pos
                while fill_pos < len(filler_steps) and filler_steps[fill_pos][0] <= h:
                    filler_steps[fill_pos][1]()
                    fill_pos += 1

            def pop_filler(n):
                nonlocal fill_pos
                k = 0
                while k < n and fill_pos < len(filler_steps):
                    filler_steps[fill_pos][1]()
                    fill_pos += 1
                    k += 1

            # ---- one attention head ----
            def head(h, fill_per_batch):
                t, r = h // 2, (h % 2) * DH
                flush_fillers(h)
                pT = wp.tile([P, NT, NQ], BF16, tag="pT", bufs=2, name=f"pT{h}")
                for cp2 in range(NT // 2):
                    s_ps = pp.tile([P, 2, NQ], F32, tag="s", bufs=2,
                                   name=f"s{h}_{cp2}")
                    for j in range(2):
                        ct = 2 * cp2 + j
                        nc.tensor.matmul(
                            s_ps[:, j, :],
                            lhsT=k_sb[r:r + DH, t, ct * P:(ct + 1) * P],
                            rhs=q_sb[r:r + DH, t, :],
                            start=True, stop=True,
                        )
                    nc.scalar.activation(pT[:, 2 * cp2:2 * cp2 + 2, :],
                                         s_ps[:, :, :], AF.Exp, scale=SCALE)
                    pop_filler(fill_per_batch[cp2])
                flush_fillers(h + 0.5)
                pv_ps = pp.tile([P, NQ], F32, tag="pv", bufs=1, name=f"pv{h}")
                for c in range(NT):
                    nc.tensor.matmul(
                        pv_ps[:, :],
                        lhsT=v_sb[:, c, h, :],
                        rhs=pT[:, c, :],
                        start=(c == 0), stop=(c == NT - 1),
                    )
                # rows 0:64 of pv_ps hold the softmax denominator replicated
                # 64x (ones block leads the stationary tile -- the custom DVE
                # reciprocal requires its input at partition base 0 on HW);
                # rows 64:128 are the PV values. Normalize with two DVE ops.
                srec = wp.tile([DH, NQ], F32, tag="srec", bufs=2, name=f"sr{h}")
                nc.vector.reciprocal_approx_fast(srec[:, :], pv_ps[0:DH, :])
                nc.vector.tensor_mul(aT_sb[r:r + DH, t, :], pv_ps[DH:P, :],
                                     srec[:, :])

            # ---- schedule ----
            bias_sb = cp.tile([P, 2, NQ], F32)
            b_ps = pp.tile([P, 2, NQ], F32, tag="s", bufs=2, name="biasps")
            for j in range(2):
                nc.tensor.matmul(b_ps[:, j, :], lhsT=ones128[:, :],
                                 rhs=bout_sb[:, j * NQ:(j + 1) * NQ],
                                 start=True, stop=True)
            nc.vector.tensor_copy(bias_sb[:, :, :], b_ps[:, :, :])
            q_proj(0)
            for s in k_steps(0):
                s()
            for nt in range(NT):
                for s in v_steps(nt, 0):
                    s()
            pacing = {c: 4 for c in range(NT // 2)}
            pacing_late = {c: 3 for c in range(NT // 2)}
            for h in range(H):
                head(h, pacing if h < 8 else pacing_late)

            # ---- output projection: 4 token-tiles of [128, 1024] ----
            for tt in range(NQ // P):
                y_ps = pp.tile([P, 2, NQ], F32, tag="s", bufs=2, name=f"yps{tt}")
                for c in range(DC):
                    for j in range(2):
                        nc.tensor.matmul(
                            y_ps[:, j, :],
                            lhsT=aT_sb[:, c, tt * P:(tt + 1) * P],
                            rhs=wout_sb[:, c, j * NQ:(j + 1) * NQ],
                            start=(c == 0), stop=(c == DC - 1),
                            skip_group_check=True,
                        )
                y_sb = wp.tile([P, D], F32, tag="y_sb", bufs=2, name=f"ysb{tt}")
                if tt < NQ // P - 1:
                    nc.vector.tensor_add(
                        y_sb[:, :].rearrange("p (g e) -> p g e", g=2),
                        y_ps[:, :, :], bias_sb[:, :, :])
                    nc.sync.dma_start(y.ap()[tt * P:(tt + 1) * P, :], y_sb[:, :])
                else:
                    for j in range(2):
                        nc.vector.tensor_add(y_sb[:, j * NQ:(j + 1) * NQ],
                                             y_ps[:, j, :], bias_sb[:, j, :])
                        nc.sync.dma_start(
                            y.ap()[tt * P:(tt + 1) * P, j * NQ:(j + 1) * NQ],
                            y_sb[:, j * NQ:(j + 1) * NQ])
    nc.compile()
    return nc


def _make_in_maps(x, w_qkv, w_out, b_out):
    import ml_dtypes
    bf = ml_dtypes.bfloat16
    wqkvT = np.ascontiguousarray(w_qkv.astype(bf).T)
    woutT = np.ascontiguousarray(w_out.astype(bf).T)
    boutr = b_out.astype(bf).reshape(1, D)
    in_maps = []
    for core in range(8):
        b, half = core // 2, core % 2
        xT = x[b].astype(bf).T
        if half:
            xT = np.concatenate([xT[:, NQ:], xT[:, :NQ]], axis=1)
        in_maps.append({
            "xkT": np.ascontiguousarray(xT),
            "wqkvT": wqkvT,
            "woutT": woutT,
            "bout": boutr,
        })
    return in_maps


def _assemble(results):
    y = np.empty((B, N, D), dtype=np.float32)
    for core in range(8):
        b, half = core // 2, core % 2
        y[b, half * NQ:(half + 1) * NQ, :] = results[core]["y"]
    return y


_NC_CACHE = {}


def kernel(x, w_qkv, w_out, b_out):
    import numpy as _np
    from concourse.bass_utils import run_bass_kernel_spmd
    if "nc" not in _NC_CACHE:
        _NC_CACHE["nc"] = _build_nc()
    nc = _NC_CACHE["nc"]
    in_maps = _make_in_maps(_np.asarray(x), _np.asarray(w_qkv),
                            _np.asarray(w_out), _np.asarray(b_out))
    res = run_bass_kernel_spmd(nc, in_maps, list(range(8)))
    return _assemble(res.results)
